# revision 1
# baseline (speedup 1.0000x reference)
"""GAT (3-layer, PyG GATConv-style) Trainium2 Bass kernel, 8-core SPMD.

Strategy (dst-sharded edge parallelism):
  - Pad N to NPAD (multiple of 1024). Core c owns node range [c*NPAD/8, (c+1)*NPAD/8),
    i.e. BPC = NPAD/1024 blocks of 128 dst nodes.
  - Host: append self-loops, sort edges by dst, assign each edge to the core that
    owns its dst, pad each (core, block) to T tiles of 128 edges. Indices/slots are
    shipped as per-core int16/bf16 tables; the device program is identical on all
    cores (same NEFF), only input data differs.
  - Per layer L: each core computes h_aug = x @ W_aug for its own nodes
    (W_aug has fused per-head attention projections a_src/a_dst as extra columns),
    stores rows [h | s_src | s_dst | pad] to DRAM, AllGather -> full table.
    Then per dst block: dma_gather rows by src (features+s_src) and by dst
    (s_dst slice); p = exp(leakyrelu(s_src+s_dst)); one-hot (edge->dst slot)
    matmuls aggregate numer = sum p*h and denom = sum p into PSUM; normalize,
    mean heads, +bias, relu -> next layer input (kept transposed in SBUF).
  - Layer 3 ends with a ones-vector matmul accumulating the node-mean partial;
    host sums the 8 per-core [1,128] partials.
"""

import numpy as np
import ml_dtypes

BF16 = ml_dtypes.bfloat16
NCORES = 8


# ----------------------------------------------------------------------------
# Host-side preprocessing
# ----------------------------------------------------------------------------

def _wrap16(idx_flat):
    """dma_gather index layout: [128, n/16] int16, idx i at [i%16, i//16],
    replicated across the 8 groups of 16 partitions."""
    n = idx_flat.shape[0]
    assert n % 16 == 0
    w = idx_flat.reshape(n // 16, 16).T.astype(np.int16)  # [16, n/16]
    return np.tile(w, (8, 1))  # [128, n/16]


def prep_static(edge_index, N, NPAD):
    """Edge structure -> per-core gather/slot tables. Returns (T, idxs, idxd, slot)."""
    E0 = edge_index.shape[1]
    loops = np.arange(N, dtype=np.int64)
    src = np.concatenate([edge_index[0].astype(np.int64), loops])
    dst = np.concatenate([edge_index[1].astype(np.int64), loops])
    order = np.argsort(dst, kind="stable")
    src_s, dst_s = src[order], dst[order]

    BPC = NPAD // (128 * NCORES)
    n_blocks = NPAD // 128
    NPC = NPAD // NCORES
    CR = 512 if NPC % 512 == 0 else NPC  # allgather chunk rows (<1MB/rank -> mesh)
    # hf row layout after chunked allgather: chunk k holds rank-c rows
    # [k*CR,(k+1)*CR) at hf rows k*CR*8 + c*CR + r%CR
    def node2row(n):
        c, r = n // NPC, n % NPC
        return (r // CR) * (CR * NCORES) + c * CR + (r % CR)
    # contiguous edge range per global block
    bounds = np.searchsorted(dst_s, np.arange(n_blocks + 1) * 128)
    counts = bounds[1:] - bounds[:-1]
    # per-block-index tile count: max over cores for that block position
    counts_cb = counts.reshape(NCORES, BPC)
    TBS = tuple(int(t) for t in np.maximum(
        1, np.ceil(counts_cb.max(axis=0) / 128).astype(np.int64)))

    idxs_cores, idxd_cores, slot_cores = [], [], []
    for c in range(NCORES):
        iw_cols, dw_cols, sl_cols = [], [], []
        for b in range(BPC):
            Tb = TBS[b]
            g = c * BPC + b
            lo, hi = int(bounds[g]), int(bounds[g + 1])
            n_e = hi - lo
            gsrc = np.zeros(Tb * 128, dtype=np.int64)
            gdst = np.zeros(Tb * 128, dtype=np.int64)
            gslot = np.full(Tb * 128, 255.0, dtype=np.float32)
            gsrc[:n_e] = node2row(src_s[lo:hi])
            gdst[:n_e] = node2row(dst_s[lo:hi])
            gslot[:n_e] = (dst_s[lo:hi] - g * 128).astype(np.float32)
            iw_cols.append(_wrap16(gsrc))
            dw_cols.append(_wrap16(gdst))
            # edge i of block -> (tile t=i//128, partition p=i%128)
            sl_cols.append(gslot.reshape(Tb, 128).T)  # [128, Tb]
        idxs_cores.append(np.concatenate(iw_cols, axis=1))
        idxd_cores.append(np.concatenate(dw_cols, axis=1))
        slot_cores.append(np.concatenate(sl_cols, axis=1).astype(np.float32))
    oh_cores = []
    for c in range(NCORES):
        sl = slot_cores[c]  # [128, sum(TBS)] float32
        oh = (sl[:, :, None] == np.arange(128, dtype=np.float32)[None, None, :])
        oh_cores.append(np.ascontiguousarray(
            oh.astype(BF16).reshape(128, -1)))  # [128, BPC*T*128]
    return TBS, idxs_cores, idxd_cores, oh_cores, CR


def prep_values(x, Ws, a_srcs, a_dsts, bs, NPAD):
    """Cast/fuse parameters. Returns dict of host arrays shared by all cores
    (except xT which is per-core sliced by the caller)."""
    N, F = x.shape
    xp = np.zeros((NPAD, F), dtype=np.float32)
    xp[:N] = x
    xT = np.ascontiguousarray(xp.T).astype(BF16)  # [F, NPAD]

    W_augs = []
    for W, a_s, a_d in zip(Ws, a_srcs, a_dsts):
        H, Fin, C = W.shape
        RW = _row_width(H, C)
        Wf = np.transpose(W, (1, 0, 2)).reshape(Fin, H * C)
        wsrc = np.einsum("hfc,hc->fh", W, a_s)
        wdst = np.einsum("hfc,hc->fh", W, a_d)
        off = H * C + (1 if H == 1 else 0)  # H==1: col H*C is the ones col
        Wa = np.zeros((Fin, RW), dtype=np.float32)
        Wa[:, : H * C] = Wf
        Wa[:, off : off + H] = wsrc
        Wa[:, off + H : off + 2 * H] = wdst
        W_augs.append(Wa.astype(BF16))
    return xT, W_augs


def _row_width(H, C):
    """h_aug row width (elements): H*C features + 2H scores, padded so the
    bf16 row is a multiple of 256 bytes (=128 elements)."""
    used = H * C + 2 * H
    return ((used + 127) // 128) * 128


# ----------------------------------------------------------------------------
# Device program
# ----------------------------------------------------------------------------

def build_nc(cfg, repeat=1):
    import concourse.bacc as bacc
    import concourse.bass as bass
    import concourse.mybir as mybir
    import concourse.tile as tile
    from concourse.masks import make_identity
    from contextlib import ExitStack

    f32 = mybir.dt.float32
    bf16 = mybir.dt.bfloat16
    i16 = mybir.dt.int16
    ALU = mybir.AluOpType
    ACT = mybir.ActivationFunctionType

    N = cfg["N"]
    NPAD = cfg["NPAD"]
    F_IN = cfg["F_IN"]
    C = cfg["C"]
    TBS = cfg["TBS"]            # tiles per block index
    SUMT = sum(TBS)
    tb_off = [0]
    for tb in TBS:
        tb_off.append(tb_off[-1] + tb)
    HS = cfg["HS"]              # heads per layer, e.g. (4, 4, 1)
    BPC = NPAD // (128 * NCORES)
    CR = cfg["CR"]
    NPC = NPAD // NCORES
    NCH = NPC // CR
    NL = len(HS)
    RWs = [_row_width(H, C) for H in HS]
    FINs = [F_IN] + [C] * (NL - 1)

    nc = bacc.Bacc("TRN2", target_bir_lowering=False, debug=False,
                   num_devices=NCORES)

    # ---- I/O ----
    xT_d = nc.dram_tensor("xT", [F_IN, NPAD // NCORES], bf16, kind="ExternalInput")
    idxs_d = nc.dram_tensor("idxs", [128, SUMT * 8], i16, kind="ExternalInput")
    idxd_d = nc.dram_tensor("idxd", [128, SUMT * 8], i16, kind="ExternalInput")
    oh_d = nc.dram_tensor("oh", [128, SUMT * 128], bf16, kind="ExternalInput")
    W_d = [nc.dram_tensor(f"w{i+1}", [FINs[i], RWs[i]], bf16, kind="ExternalInput")
           for i in range(NL)]
    bb_d = [nc.dram_tensor(f"bb{i+1}", [C, 1], f32, kind="ExternalInput")
            for i in range(NL - 1)]
    b3r_d = nc.dram_tensor("b3r", [1, C], f32, kind="ExternalInput")
    out_d = nc.dram_tensor("out", [1, C], f32, kind="ExternalOutput")

    with tile.TileContext(nc, num_cores=NCORES) as tc, ExitStack() as ctx:
        dram = ctx.enter_context(tc.tile_pool(name="dram", bufs=1, space="DRAM"))
        cpool = ctx.enter_context(tc.tile_pool(name="consts", bufs=1))
        gpool = ctx.enter_context(tc.tile_pool(name="gath", bufs=3))
        ohpool = ctx.enter_context(tc.tile_pool(name="oh", bufs=4))
        wpool = ctx.enter_context(tc.tile_pool(name="work", bufs=3))
        fpool = ctx.enter_context(tc.tile_pool(name="fin", bufs=2))
        hpool = ctx.enter_context(tc.tile_pool(name="haug", bufs=3))
        psum = ctx.enter_context(tc.tile_pool(name="ps", bufs=2, space="PSUM"))

        # DRAM scratch (pool tiles so Tile tracks collective/gather deps)
        hl = [dram.tile([NPAD // NCORES, RWs[i]], bf16, tag=f"hl{i}",
                        name=f"hl{i}") for i in range(NL)]
        hf = [dram.tile([NPAD, RWs[i]], bf16, tag=f"hf{i}", name=f"hf{i}")
              for i in range(NL)]

        # ---- constants into SBUF ----
        ident = cpool.tile([128, 128], f32, tag="ident")
        make_identity(nc, ident[:])
        xT_sb = cpool.tile([F_IN, NPAD // NCORES], bf16, tag="xT")
        nc.sync.dma_start(xT_sb[:], xT_d[:, :])
        idxs_sb = cpool.tile([128, SUMT * 8], i16, tag="idxs")
        nc.sync.dma_start(idxs_sb[:], idxs_d[:, :])
        idxd_sb = cpool.tile([128, SUMT * 8], i16, tag="idxd")
        nc.sync.dma_start(idxd_sb[:], idxd_d[:, :])
        W_sb = []
        for i in range(NL):
            w = cpool.tile([FINs[i], RWs[i]], bf16, tag=f"w{i}", name=f"w{i}")
            nc.sync.dma_start(w[:], W_d[i][:, :])
            W_sb.append(w)
        bb_sb = []
        for i in range(NL - 1):
            b = cpool.tile([C, 1], f32, tag=f"bb{i}", name=f"bb{i}")
            nc.sync.dma_start(b[:], bb_d[i][:, :])
            bb_sb.append(b)
        b3_sb = cpool.tile([1, C], f32, tag="b3")
        nc.sync.dma_start(b3_sb[:], b3r_d[:, :])
        ones_sb = cpool.tile([128, 1], f32, tag="ones")
        nc.vector.memset(ones_sb[:], 1.0)

        # next-layer transposed features, per layer boundary
        x2T = [cpool.tile([128, NPAD // NCORES], bf16, tag=f"x2T{i}",
                          name=f"x2T{i}") for i in range(NL - 1)]

        pfin = psum.tile([1, C], f32, tag="pfin", bufs=1)

        for _rep in range(repeat):
         for L in range(NL):
             H = HS[L]
             RW = RWs[L]
             SOFF = H * C + (1 if H == 1 else 0)  # s_src offset (H==1: skip ones col)
             S2 = 128                          # gather2 slice width (256B)
             s2off = (SOFF // 128) * 128       # aligned slice start covering s cols
             s_src_in2 = SOFF - s2off          # s_src position inside slice
             HC = H * C

             # ---- phase A: h_aug for own nodes ----
             for b in range(BPC):
                 if L == 0:
                     lhs = xT_sb[:, b * 128:(b + 1) * 128]
                 else:
                     lhs = x2T[L - 1][:, b * 128:(b + 1) * 128]
                 hs = hpool.tile([128, RW], bf16, tag="hs")
                 if RW > 512:
                     p1 = psum.tile([128, 512], f32, tag="pnum")
                     nc.tensor.matmul(p1[:], lhs, W_sb[L][:, 0:512],
                                      start=True, stop=True)
                     p2 = psum.tile([128, RW - 512], f32, tag="p128")
                     nc.tensor.matmul(p2[:], lhs, W_sb[L][:, 512:RW],
                                      start=True, stop=True)
                     nc.scalar.copy(hs[:, 0:512], p1[:])
                     nc.vector.tensor_copy(hs[:, 512:RW], p2[:])
                 else:
                     p1 = psum.tile([128, RW], f32, tag="pnum")
                     nc.tensor.matmul(p1[:], lhs, W_sb[L][:, 0:RW],
                                      start=True, stop=True)
                     nc.scalar.copy(hs[:, 0:RW], p1[:])
                 if H == 1:
                     nc.vector.memset(hs[:, HC:HC + 1], 1.0)
                 nc.sync.dma_start(hl[L][b * 128:(b + 1) * 128, :], hs[:])

             # ---- phase B: allgather, chunked <1MB/rank to stay on mesh algo ----
             for k in range(NCH):
                 nc.gpsimd.collective_compute(
                     "AllGather", mybir.AluOpType.bypass,
                     replica_groups=[list(range(NCORES))],
                     ins=[hl[L][k * CR:(k + 1) * CR, :].opt()],
                     outs=[hf[L][k * CR * NCORES:(k + 1) * CR * NCORES, :].opt()],
                 )

             # ---- phase C: edge aggregation per dst block ----
             GC = 6  # tiles per gather chunk (768 idxs = 48 desc/engine <= 64-desc packet limit)
             for b in range(BPC):
                 T = TBS[b]
                 base = tb_off[b]
                 chunks = [(c0, min(GC, T - c0)) for c0 in range(0, T, GC)]
                 NW = HC + 1 if H == 1 else HC  # H==1: denom rides as col C
                 numer = psum.tile([128, NW], f32, tag="pnum")
                 if H > 1:
                     denom = psum.tile([128, H], f32, tag="pden")
                 g1s, g2s = [], []
                 sc = wpool.tile([128, T, H], f32, tag="sc")
                 ohc = ohpool.tile([128, T * 128], bf16, tag="ohc", bufs=2)
                 nc.sync.dma_start(
                     ohc[:], oh_d[:, base * 128:(base + T) * 128])
                 for c0, tc_n in chunks:
                     ic = slice((base + c0) * 8, (base + c0 + tc_n) * 8)
                     g1 = gpool.tile([128, tc_n, RW], bf16, tag="g1", bufs=8)
                     nc.gpsimd.dma_gather(g1[:], hf[L][:, :], idxs_sb[:, ic],
                                          tc_n * 128, tc_n * 128, RW)
                     g2 = gpool.tile([128, tc_n, S2], bf16, tag="g2", bufs=8)
                     nc.gpsimd.dma_gather(g2[:], hf[L][:, s2off:s2off + S2],
                                          idxd_sb[:, ic], tc_n * 128, tc_n * 128,
                                          S2, elem_step=RW)
                     nc.vector.tensor_tensor(
                         sc[:, c0:c0 + tc_n, :], g1[:, :, SOFF:SOFF + H],
                         g2[:, :, s_src_in2 + H:s_src_in2 + 2 * H], ALU.add)
                     g1s.append(g1); g2s.append(g2)

                 # p = exp(leakyrelu(sc)), batched per block [128, T, H]
                 lr = wpool.tile([128, T, H], f32, tag="lr")
                 nc.vector.tensor_scalar(lr[:], sc[:], 0.2, None, op0=ALU.mult)
                 lr2 = wpool.tile([128, T, H], f32, tag="lr2")
                 nc.vector.tensor_tensor(lr2[:], lr[:], sc[:], ALU.max)
                 p = wpool.tile([128, T, H], f32, tag="p")
                 nc.scalar.activation(p[:], lr2[:], ACT.Exp)
                 if H > 1:
                     pb = wpool.tile([128, T, H], bf16, tag="pb")
                     nc.vector.tensor_copy(pb[:], p[:])

                 for ci, (c0, tc_n) in enumerate(chunks):
                     g1 = g1s[ci]
                     for tt in range(tc_n):
                         t = c0 + tt
                         oh_ap = ohc[:, t * 128:(t + 1) * 128]
                         msg = wpool.tile([128, NW], bf16, tag="msg")
                         if H == 1:
                             # one mul over [h | ones] -> [p*h | p]; one matmul
                             nc.vector.tensor_scalar(
                                 msg[:], g1[:, tt, 0:NW], p[:, t, 0:1],
                                 None, op0=ALU.mult)
                         else:
                             for h in range(H):
                                 src_ap = g1[:, tt, h * C:(h + 1) * C]
                                 dst_ap = msg[:, h * C:(h + 1) * C]
                                 pcol = p[:, t, h:h + 1]
                                 if h % 2 == 0:
                                     nc.vector.tensor_scalar(dst_ap, src_ap,
                                                             pcol, None,
                                                             op0=ALU.mult)
                                 else:
                                     nc.scalar.mul(dst_ap, src_ap, pcol)
                         nc.tensor.matmul(numer[:], oh_ap, msg[:],
                                          start=(t == 0), stop=(t == T - 1))
                         if H > 1:
                             nc.tensor.matmul(denom[:], oh_ap, pb[:, t, :],
                                              start=(t == 0), stop=(t == T - 1))

                 # ---- finalize block ----
                 dn = fpool.tile([128, H], f32, tag="dn")
                 dsrc = denom[:] if H > 1 else numer[:, HC:HC + 1]
                 nc.vector.tensor_scalar(dn[:], dsrc, float(H), 1e-16 * H,
                                         op0=ALU.mult, op1=ALU.add)
                 rc = fpool.tile([128, H], f32, tag="rc")
                 nc.vector.reciprocal(rc[:], dn[:])
                 if L < NL - 1:
                     ms = []
                     for h in range(H):
                         m = fpool.tile([128, C], f32, tag=f"m{h}", name=f"m{h}")
                         if h % 2 == 0:
                             nc.vector.tensor_scalar(
                                 m[:], numer[:, h * C:(h + 1) * C],
                                 rc[:, h:h + 1], None, op0=ALU.mult)
                         else:
                             nc.scalar.mul(m[:], numer[:, h * C:(h + 1) * C],
                                           rc[:, h:h + 1])
                         ms.append(m)
                     acc = ms[0]
                     if H > 1:
                         s01 = fpool.tile([128, C], f32, tag="s01")
                         nc.vector.tensor_tensor(s01[:], ms[0][:], ms[1][:], ALU.add)
                         acc = s01
                         if H == 4:
                             s23 = fpool.tile([128, C], f32, tag="s23")
                             nc.vector.tensor_tensor(s23[:], ms[2][:], ms[3][:],
                                                     ALU.add)
                             s4 = fpool.tile([128, C], f32, tag="s4")
                             nc.vector.tensor_tensor(s4[:], s01[:], s23[:], ALU.add)
                             acc = s4
                     pt = psum.tile([128, 128], f32, tag="p128")
                     nc.tensor.transpose(pt[:], acc[:], ident[:])
                     nc.scalar.activation(x2T[L][:, b * 128:(b + 1) * 128],
                                          pt[:], ACT.Relu, bias=bb_sb[L][:])
                 else:
                     o3 = fpool.tile([128, C], f32, tag="o3")
                     nc.vector.tensor_scalar(o3[:], numer[:, 0:C], rc[:, 0:1],
                                             None, op0=ALU.mult)
                     nc.tensor.matmul(pfin[:], ones_sb[:], o3[:],
                                      start=(b == 0), stop=(b == BPC - 1))

        fs = fpool.tile([1, C], f32, tag="fs")
        nc.vector.tensor_scalar(fs[:], pfin[:], 1.0 / N, None, op0=ALU.mult)
        fs2 = fpool.tile([1, C], f32, tag="fs2")
        nc.vector.tensor_tensor(fs2[:], fs[:], b3_sb[:], ALU.add)
        nc.sync.dma_start(out_d[:, :], fs2[:])

    nc.compile()
    return nc


# ----------------------------------------------------------------------------
# Entry points
# ----------------------------------------------------------------------------

def make_cfg_and_maps(inputs):
    x = np.asarray(inputs["x"])
    edge_index = np.asarray(inputs["edge_index"])
    N, F_IN = x.shape
    NPAD = ((N + 1023) // 1024) * 1024
    Ws = [np.asarray(inputs[f"W{i}"]) for i in (1, 2, 3)]
    a_srcs = [np.asarray(inputs[f"as{i}"]) for i in (1, 2, 3)]
    a_dsts = [np.asarray(inputs[f"ad{i}"]) for i in (1, 2, 3)]
    bs = [np.asarray(inputs[f"b{i}"]) for i in (1, 2, 3)]
    HS = tuple(W.shape[0] for W in Ws)
    C = Ws[0].shape[2]

    TBS, idxs_c, idxd_c, oh_c, CR = prep_static(edge_index, N, NPAD)
    xT, W_augs = prep_values(x, Ws, a_srcs, a_dsts, bs, NPAD)

    cfg = dict(N=N, NPAD=NPAD, F_IN=F_IN, C=C, TBS=TBS, HS=HS, CR=CR)
    NPC = NPAD // NCORES
    in_maps = []
    for c in range(NCORES):
        m = {
            "xT": np.ascontiguousarray(xT[:, c * NPC:(c + 1) * NPC]),
            "idxs": idxs_c[c],
            "idxd": idxd_c[c],
            "oh": oh_c[c],
            "b3r": (bs[2] * (1.0 / NCORES)).reshape(1, C).astype(np.float32),
        }
        for i in range(3):
            m[f"w{i+1}"] = W_augs[i]
        for i in range(2):
            m[f"bb{i+1}"] = bs[i].astype(np.float32).reshape(C, 1)
        in_maps.append(m)
    return cfg, in_maps


_NC_CACHE = {}


def _get_nc(cfg, repeat=1):
    key = (repeat,) + tuple(sorted((k, v if not isinstance(v, tuple) else v)
                                   for k, v in cfg.items()))
    if key not in _NC_CACHE:
        _NC_CACHE[key] = build_nc(cfg, repeat=repeat)
    return _NC_CACHE[key]


def run(inputs, trace=False, repeat=1, **kw):
    from concourse.bass_utils import run_bass_kernel_spmd
    cfg, in_maps = make_cfg_and_maps(inputs)
    nc = _get_nc(cfg, repeat=repeat)
    res = run_bass_kernel_spmd(nc, in_maps, core_ids=list(range(NCORES)),
                               trace=trace, **kw)
    out = np.zeros((1, cfg["C"]), dtype=np.float32)
    for r in res.results:
        out += r["out"]
    return out, res


def kernel(**inputs) -> np.ndarray:
    out, _ = run(inputs)
    return out



# revision 3
# speedup vs baseline: 1.1216x; 1.1216x over previous
"""GAT (3-layer, PyG GATConv-style) Trainium2 Bass kernel, 8-core SPMD.

Instruction-count-minimized redesign (the axon path serializes at ~60us per
instruction, so wall time ~ total instruction count):

  - Nodes are sorted by in-degree and dealt round-robin to the 8 cores, so
    all cores share one compile-time block structure with near-identical
    per-slot degrees. dst-sharded edge parallelism as before.
  - Per layer, each core computes h_aug rows [h(H*C) | s_src(H)] for its own
    nodes (2 matmuls / 128 nodes), plus a transposed s_dst table [H, slots]
    (5 matmuls), AllGathers the row table to a full DRAM table, then
    aggregates per dst block entirely with wide vector ops:
      * transposed dma_gather pulls the src rows feature-transposed:
        g1[p, k, g, e] = row[idx[e]][128*g + p]  (<=896 idxs per gather)
      * scores e = s_src + s_dst, Prelu(0.2), Exp run on H partitions for a
        whole block per instruction; segment-sum over the fixed per-block
        degree D happens in a single strided tensor_reduce
      * alpha = (p/H) / denom is broadcast to 128 partitions (one SBUF DMA +
        one partition_broadcast), multiplied into the gathered features
        in-place, and a single 5-dim reduce produces the head-averaged
        numerator [C, nd] directly in next-layer-transposed layout.
  - Padding edges point at a sentinel row with s_src = -1e30 (p = 0);
    degree-0 pad slots get one edge to an all-zero neutral row (p > 0,
    h = 0) so denominators stay positive.
  - Layer 3 accumulates the node-sum per block; host sums cores' partials,
    divides by N and adds b3.
"""

import numpy as np
import ml_dtypes

BF16 = ml_dtypes.bfloat16
NCORES = 8

# chunk (= one multi-packet transposed gather) is Dc*nd idxs: <= 2688, %128
def _menu():
    out = []
    for nd in (64, 96, 128, 160, 192, 224, 256, 320, 384, 448, 512):
        for dc in range(2688 // nd, 0, -1):
            if (dc * nd) % 128 == 0:
                out.append((nd, dc))
                break
    return out
MENU = _menu()
CAPE = 5376          # max edges (idx slots) per block, SBUF-driven
SENT_OFF = 0         # sentinel row = NPAD + 0
NEUT_OFF = 1         # neutral row  = NPAD + 1


def _wrap16(idx_flat):
    n = idx_flat.shape[0]
    assert n % 16 == 0
    w = idx_flat.reshape(n // 16, 16).T.astype(np.int16)
    return np.tile(w, (8, 1))


def prep_static(edge_index, N):
    """Degree-sorted node permutation + shared block structure + per-core
    gather index tables."""
    E0 = edge_index.shape[1]
    loops = np.arange(N, dtype=np.int64)
    src = np.concatenate([edge_index[0].astype(np.int64), loops])
    dst = np.concatenate([edge_index[1].astype(np.int64), loops])
    deg = np.bincount(dst, minlength=N)

    order = np.argsort(-deg, kind="stable")
    node_core = np.empty(N, dtype=np.int64)
    node_slot = np.empty(N, dtype=np.int64)
    node_core[order] = np.arange(N) % NCORES
    node_slot[order] = np.arange(N) // NCORES
    NSLOT = (N + NCORES - 1) // NCORES          # 2500
    NPC = ((NSLOT + 127) // 128) * 128          # 2560 slots incl pads
    NPAD = NPC * NCORES

    # per (core, slot) degree; Dmax over cores per slot
    slotdeg = np.zeros((NCORES, NPC), dtype=np.int64)
    slotdeg[node_core, node_slot] = deg
    Dmax = slotdeg.max(axis=0)

    # greedy shared block structure over slots
    blocks = []
    s = 0
    while s < NSLOT:
        Dneed = max(int(Dmax[s]), 1)
        pick = None
        for nd, Dc in sorted(MENU, reverse=True):   # largest nd first
            if nd > NPC - s:
                continue
            K = max(1, -(-Dneed // Dc))
            if K * Dc * nd <= CAPE:
                pick = (nd, Dc, K)
                break
        assert pick is not None, f"no block fits at slot {s}"
        blocks.append((s,) + pick)
        s += pick[0]
    BLKS = tuple(blocks)

    # hf row of original node j (allgather is chunked by CR rows: chunk ck
    # holds rank c's rows [ck*CR,(ck+1)*CR) at ck*CR*NCORES + c*CR + r%CR)
    CR = 512
    hfrow = (node_slot // CR) * (CR * NCORES) + node_core * CR +         (node_slot % CR)

    # per-core per-slot src lists (ordered by slot)
    ecore = node_core[dst]
    eslot = node_slot[dst]
    eorder = np.argsort(ecore * NPC + eslot, kind="stable")
    src_s = src[eorder]
    key_s = (ecore * NPC + eslot)[eorder]
    bounds = np.searchsorted(key_s, np.arange(NCORES * NPC + 1))

    SENT = NPAD + SENT_OFF
    NEUT = NPAD + NEUT_OFF
    idx_cores = []
    for c in range(NCORES):
        cols = []
        for (s0, nd, Dc, K) in BLKS:
            tab = np.full((K * Dc, nd), SENT, dtype=np.int64)
            for n in range(nd):
                g = c * NPC + s0 + n
                lo, hi = int(bounds[g]), int(bounds[g + 1])
                dn = hi - lo
                if dn == 0:
                    tab[0, n] = NEUT
                else:
                    tab[:dn, n] = hfrow[src_s[lo:hi]]
            for k in range(K):
                cols.append(_wrap16(tab[k * Dc:(k + 1) * Dc].reshape(-1)))
        idx_cores.append(np.concatenate(cols, axis=1))
    return BLKS, idx_cores, node_core, node_slot, NPC, NPAD


def prep_values(x, Ws, a_srcs, a_dsts, node_core, node_slot, NPC):
    N, F = x.shape
    xT_cores = np.zeros((NCORES, F, NPC), dtype=np.float32)
    xT_cores[node_core, :, node_slot] = x          # fancy: [N, F] into [c][:,s]
    xT_cores = xT_cores.astype(BF16)

    W_augs, wdsts = [], []
    for W, a_s, a_d in zip(Ws, a_srcs, a_dsts):
        H, Fin, C = W.shape
        RW = H * C + 128
        Wf = np.transpose(W, (1, 0, 2)).reshape(Fin, H * C)
        wsrc = np.einsum("hfc,hc->fh", W, a_s)
        wdst = np.einsum("hfc,hc->fh", W, a_d)
        Wa = np.zeros((Fin, RW), dtype=np.float32)
        Wa[:, :H * C] = Wf
        Wa[:, H * C:H * C + H] = wsrc
        W_augs.append(Wa.astype(BF16))
        wdsts.append(wdst.astype(BF16))
    return xT_cores, W_augs, wdsts


# ----------------------------------------------------------------------------
# Device program
# ----------------------------------------------------------------------------

def build_nc(cfg, repeat=1):
    import concourse.bacc as bacc
    import concourse.mybir as mybir
    import concourse.tile as tile
    from contextlib import ExitStack

    f32 = mybir.dt.float32
    bf16 = mybir.dt.bfloat16
    i16 = mybir.dt.int16
    ALU = mybir.AluOpType
    ACT = mybir.ActivationFunctionType
    AX = mybir.AxisListType

    N = cfg["N"]
    NPC = cfg["NPC"]
    NPAD = NPC * NCORES
    F_IN = cfg["F_IN"]
    C = cfg["C"]
    HS = cfg["HS"]
    BLKS = cfg["BLKS"]
    NB = len(BLKS)
    NL = len(HS)
    RWs = [HS[i] * C + 128 for i in range(NL)]
    FINs = [F_IN] + [C] * (NL - 1)
    SUMI = sum(K * Dc * nd for (_, nd, Dc, K) in BLKS)
    NBA = NPC // 128                      # phase-A 128-node blocks

    nc = bacc.Bacc("TRN2", target_bir_lowering=False, debug=False,
                   num_devices=NCORES)

    xT_d = nc.dram_tensor("xT", [F_IN, NPC], bf16, kind="ExternalInput")
    idx_d = nc.dram_tensor("idx", [128, SUMI // 16], i16, kind="ExternalInput")
    W_d = [nc.dram_tensor(f"w{i+1}", [FINs[i], RWs[i]], bf16,
                          kind="ExternalInput") for i in range(NL)]
    wd_d = [nc.dram_tensor(f"wd{i+1}", [FINs[i], HS[i]], bf16,
                           kind="ExternalInput") for i in range(NL)]
    bb_d = [nc.dram_tensor(f"bb{i+1}", [C, 1], f32, kind="ExternalInput")
            for i in range(NL - 1)]
    out_d = nc.dram_tensor("out", [C, 1], f32, kind="ExternalOutput")

    with tile.TileContext(nc, num_cores=NCORES) as tc, ExitStack() as ctx:
        dram = ctx.enter_context(tc.tile_pool(name="dram", bufs=1, space="DRAM"))
        cpool = ctx.enter_context(tc.tile_pool(name="consts", bufs=1))
        hpool = ctx.enter_context(tc.tile_pool(name="haug", bufs=1))
        gpool = ctx.enter_context(tc.tile_pool(name="gath", bufs=1))
        wpool = ctx.enter_context(tc.tile_pool(name="work", bufs=1))
        apool = ctx.enter_context(tc.tile_pool(name="alpha", bufs=1))
        fpool = ctx.enter_context(tc.tile_pool(name="fin", bufs=1))
        psum = ctx.enter_context(tc.tile_pool(name="ps", bufs=2, space="PSUM"))

        hl = [dram.tile([NPC, RWs[i]], bf16, tag=f"hl{i}", name=f"hl{i}")
              for i in range(NL)]
        hf = [dram.tile([NPAD + 128, RWs[i]], bf16, tag=f"hf{i}",
                        name=f"hf{i}") for i in range(NL)]

        # ---- constants ----
        xT_sb = cpool.tile([F_IN, NPC], bf16, tag="xT")
        nc.sync.dma_start(xT_sb[:], xT_d[:, :])
        idx_sb = cpool.tile([128, SUMI // 16], i16, tag="idx")
        nc.sync.dma_start(idx_sb[:], idx_d[:, :])
        W_sb, wd_sb, bb_sb = [], [], []
        for i in range(NL):
            w = cpool.tile([FINs[i], RWs[i]], bf16, tag=f"w{i}", name=f"w{i}")
            nc.sync.dma_start(w[:], W_d[i][:, :])
            W_sb.append(w)
            wd = cpool.tile([FINs[i], HS[i]], bf16, tag=f"wd{i}", name=f"wd{i}")
            nc.sync.dma_start(wd[:], wd_d[i][:, :])
            wd_sb.append(wd)
        for i in range(NL - 1):
            b = cpool.tile([C, 1], f32, tag=f"bb{i}", name=f"bb{i}")
            nc.sync.dma_start(b[:], bb_d[i][:, :])
            bb_sb.append(b)

        # sentinel (s_src = -1e30) + neutral (all zero) rows per layer table
        for L in range(NL):
            HC = HS[L] * C
            srow = cpool.tile([1, 2, RWs[L]], bf16, tag=f"sr{L}", name=f"sr{L}")
            nc.vector.memset(srow[:], 0.0)
            nc.vector.memset(srow[:, 0, HC:HC + HS[L]], -1e30)
            nc.sync.dma_start(hf[L][NPAD:NPAD + 2, :], srow[:])

        # next-layer transposed features (phase C writes, phase A reads)
        x2T = [cpool.tile([C, NPC], bf16, tag=f"x2T{i}", name=f"x2T{i}")
               for i in range(NL - 1)]
        for t in x2T:
            nc.vector.memset(t[:], 0.0)

        nout = cpool.tile([C, NPC], f32, tag="nout")
        nc.vector.memset(nout[:], 0.0)

        # hoisted num_idxs registers (avoid one RegisterMove per gather)
        cnds = sorted({Dc * nd for (_, nd, Dc, K) in BLKS})
        cnd_reg = {v: nc.gpsimd.to_reg(v) for v in cnds}

        for _rep in range(repeat):
            for L in range(NL):
                H = HS[L]
                RW = RWs[L]
                G = RW // 128
                HC = H * C
                xin = xT_sb if L == 0 else x2T[L - 1]

                # ---- phase A: h_aug rows for own slots ----
                for nb0 in range(0, NBA, 4):
                    nb1 = min(nb0 + 4, NBA)
                    nw = nb1 - nb0
                    hs = hpool.tile([128, 4, RW], bf16, tag="hs")
                    for nb in range(nb0, nb1):
                        lhs = xin[:, nb * 128:(nb + 1) * 128]
                        j = nb - nb0
                        if RW > 512:
                            p1 = psum.tile([128, 640], f32, tag="pA")
                            nc.tensor.matmul(p1[:, 0:512], lhs, W_sb[L][:, 0:512],
                                             start=True, stop=True)
                            nc.tensor.matmul(p1[:, 512:RW], lhs, W_sb[L][:, 512:RW],
                                             start=True, stop=True)
                            nc.scalar.copy(hs[:, j, 0:RW], p1[:, 0:RW])
                        else:
                            p1 = psum.tile([128, RW], f32, tag="pA")
                            nc.tensor.matmul(p1[:], lhs, W_sb[L][:, 0:RW],
                                             start=True, stop=True)
                            nc.scalar.copy(hs[:, j, 0:RW], p1[:])
                    # hl[nb0*128 + j*128 + p] = hs[p, j, :]
                    orows = hl[L][nb0 * 128:nb1 * 128, :].rearrange(
                        "(j p) w -> p j w", j=nw)
                    nc.sync.dma_start(orows, hs[:, 0:nw, :])

                # ---- transposed s_dst for own slots: [H, NPC] ----
                sdT = cpool.tile([HS[L], NPC], bf16, tag=f"sdT{L}",
                                 name=f"sdT{L}")
                for j in range(NPC // 512):
                    ps = psum.tile([HS[L], 512], f32, tag="pS")
                    nc.tensor.matmul(ps[:], wd_sb[L],
                                     xin[:, j * 512:(j + 1) * 512],
                                     start=True, stop=True)
                    nc.scalar.copy(sdT[:, j * 512:(j + 1) * 512], ps[:])

                # ---- allgather ----
                CR = 512
                for ck in range(NPC // CR):
                    nc.gpsimd.collective_compute(
                        "AllGather", mybir.AluOpType.bypass,
                        replica_groups=[list(range(NCORES))],
                        ins=[hl[L][ck * CR:(ck + 1) * CR, :].opt()],
                        outs=[hf[L][ck * CR * NCORES:(ck + 1) * CR * NCORES,
                                    :].opt()],
                    )

                # ---- phase C ----
                col = 0
                for bi, (s0, nd, Dc, K) in enumerate(BLKS):
                    CND = Dc * nd
                    g1 = gpool.tile([128, K, G, CND], bf16, tag="g1")
                    for k in range(K):
                        nc.gpsimd.dma_gather(
                            g1[:, k, :, :], hf[L][:, :],
                            idx_sb[:, col:col + CND // 16],
                            CND, cnd_reg[CND], RW, transpose=True,
                            single_packet=False)
                        col += CND // 16

                    # scores -> p (in place), on H partitions
                    e = wpool.tile([H, K, Dc, nd], f32, tag="e")
                    ssrc = g1[0:H, :, G - 1, :].rearrange(
                        "p k (d n) -> p k d n", d=Dc)
                    sd_v = sdT[:, s0:s0 + nd].unsqueeze(1).unsqueeze(1) \
                        .broadcast_to((H, K, Dc, nd))
                    nc.vector.tensor_tensor(e[:], ssrc, sd_v, ALU.add)
                    e_3 = e[:].rearrange("p k d n -> p (k d) n")
                    nc.vector.scalar_tensor_tensor(e_3, e_3, 0.2, e_3,
                                                   op0=ALU.mult, op1=ALU.max)
                    nc.scalar.activation(e[:], e[:], ACT.Exp)

                    den = wpool.tile([H, nd], f32, tag="den")
                    nc.vector.tensor_reduce(den[:], e[:].transpose([0, 3, 1, 2]),
                                            AX.XY, ALU.add)
                    rc = wpool.tile([H, nd], f32, tag="rc")
                    nc.vector.reciprocal(rc[:], den[:])
                    al = wpool.tile([H, K, Dc, nd], bf16, tag="al")
                    rc_v = rc[:].unsqueeze(1).broadcast_to((H, K * Dc, nd))
                    e_f = e[:].rearrange("p k d n -> p (k d) n")
                    al_f = al[:].rearrange("p k d n -> p (k d) n")
                    nc.vector.scalar_tensor_tensor(al_f, e_f, 1.0 / H, rc_v,
                                                   op0=ALU.mult, op1=ALU.mult)

                    # broadcast alpha to all 128 partitions
                    aa = apool.tile([128, H, K * CND], bf16, tag="aa")
                    if H > 1:
                        adr = dram.tile([H, K * CND], bf16, tag="adr",
                                        name="adr")
                        nc.sync.dma_start(adr[:], al[:])
                        nc.sync.dma_start(
                            aa[:], adr[:].unsqueeze(0).broadcast_to(
                                (128, H, K * CND)))
                    else:
                        nc.gpsimd.partition_broadcast(aa[:], al[0:1, :, :, :])

                    # msg = h * alpha, in place on gathered feature groups
                    g1f = g1[:, :, 0:H, :]
                    aa_v = aa[:].rearrange("p h (k c) -> p k h c", k=K)
                    nc.vector.tensor_tensor(g1f, g1f, aa_v, ALU.mult)

                    # numer + head mean: [128, nd] in transposed layout
                    if L < NL - 1:
                        num_t = fpool.tile([C, nd], f32, tag="num", name="num_t")
                        num_ap = num_t[:]
                    else:
                        num_ap = nout[:, s0:s0 + nd]
                    if H > 1:
                        nv = g1[:, :, 0:H, :].rearrange(
                            "p k h (d n) -> p n h k d", d=Dc)
                        nc.vector.tensor_reduce(num_ap, nv, AX.XYZ, ALU.add)
                    else:
                        nv = g1[:, :, 0, :].rearrange(
                            "p k (d n) -> p n k d", d=Dc)
                        nc.vector.tensor_reduce(num_ap, nv, AX.XY, ALU.add)

                    if L < NL - 1:
                        nc.vector.tensor_scalar(x2T[L][:, s0:s0 + nd], num_ap,
                                                bb_sb[L][:], 0.0,
                                                op0=ALU.add, op1=ALU.max)

        fo = fpool.tile([C, 1], f32, tag="fo")
        nc.vector.tensor_reduce(fo[:], nout[:], AX.X, ALU.add)
        nc.sync.dma_start(out_d[:, :], fo[:])

    nc.compile()
    return nc


# ----------------------------------------------------------------------------
# Entry points
# ----------------------------------------------------------------------------

def make_cfg_and_maps(inputs):
    x = np.asarray(inputs["x"])
    edge_index = np.asarray(inputs["edge_index"])
    N, F_IN = x.shape
    Ws = [np.asarray(inputs[f"W{i}"]) for i in (1, 2, 3)]
    a_srcs = [np.asarray(inputs[f"as{i}"]) for i in (1, 2, 3)]
    a_dsts = [np.asarray(inputs[f"ad{i}"]) for i in (1, 2, 3)]
    bs = [np.asarray(inputs[f"b{i}"]) for i in (1, 2, 3)]
    HS = tuple(W.shape[0] for W in Ws)
    C = Ws[0].shape[2]

    BLKS, idx_cores, node_core, node_slot, NPC, NPAD = \
        prep_static(edge_index, N)
    xT_cores, W_augs, wdsts = prep_values(
        x, Ws, a_srcs, a_dsts, node_core, node_slot, NPC)

    cfg = dict(N=N, NPC=NPC, F_IN=F_IN, C=C, HS=HS, BLKS=BLKS)
    in_maps = []
    for c in range(NCORES):
        m = {
            "xT": np.ascontiguousarray(xT_cores[c]),
            "idx": idx_cores[c],
        }
        for i in range(3):
            m[f"w{i+1}"] = W_augs[i]
            m[f"wd{i+1}"] = wdsts[i]
        for i in range(2):
            m[f"bb{i+1}"] = bs[i].astype(np.float32).reshape(C, 1)
        in_maps.append(m)
    return cfg, in_maps, bs[2]


_NC_CACHE = {}


def _get_nc(cfg, repeat=1):
    key = (repeat, cfg["N"], cfg["NPC"], cfg["F_IN"], cfg["C"], cfg["HS"],
           cfg["BLKS"])
    if key not in _NC_CACHE:
        _NC_CACHE[key] = build_nc(cfg, repeat=repeat)
    return _NC_CACHE[key]


def run(inputs, trace=False, repeat=1, **kw):
    from concourse.bass_utils import run_bass_kernel_spmd
    cfg, in_maps, b3 = make_cfg_and_maps(inputs)
    nc = _get_nc(cfg, repeat=repeat)
    res = run_bass_kernel_spmd(nc, in_maps, core_ids=list(range(NCORES)),
                               trace=trace, **kw)
    acc = np.zeros((cfg["C"],), dtype=np.float32)
    for r in res.results:
        acc += r["out"].reshape(-1)
    out = (acc / cfg["N"] + b3.astype(np.float32)).reshape(1, cfg["C"])
    return out, res


def kernel(**inputs) -> np.ndarray:
    out, _ = run(inputs)
    return out
